# revision 1
# baseline (speedup 1.0000x reference)
"""Trainium2 Bass kernel for DirectVolumeRenderer (axis-aligned camera).

Factorization (per depth p, camera R=I so sample coords are separable):
    trilinear(vol) = z-lerp of 2 slices -> two matmuls with the SAME tent
    matrix  A_p[v,q] = relu(1 - |v - (a_p + s_p*q)|):
        T1   = Zp^T @ A_p          (contract y)
        feat = A_p^T @ T1          (contract x) -> image in [px,py] layout
    sigma_p = 0.1*az_p * av_p[px] (x) av_p[py]  (rank-1, host vectors)

Key simplification: transmittance Gamma_k is DATA-INDEPENDENT (density is
a constant 0.1 and the ray/volume geometry is fixed).  On sigma_k's
support (the nested valid square S_k) every earlier sigma_j was fully
inside its own square, so Gamma_k == gamma_k = prod_{j<k}(1 - 0.1*az_j),
a host-computable SCALAR (validated to ~3e-6 against the exact 2D
recurrence).  The device therefore computes only
    rgb = sum_k (gamma_k * sigma_k) .* feat_k
with gamma_k folded into the host-side sigma u-vectors -- no serial
compositing chain on the device at all.

Depths with gamma < 4e-5 are truncated (96 of 240 remain; 5.7e-5 rel
err), sharded as 8 contiguous runs of 12 per core.  Each core writes its
partial image; kernel() sums the 8 partials and normalizes on the host
as the gather/unshard step (the NRT collective path costs a fixed
~55-75us bootstrap barrier + AllReduce, dominating everything else —
see kernel_v4.py for the on-device fp16-AllReduce variant at ~92us).

Per depth: PE does mm1/mm2 (bf16, tents and sigma fields streamed from
host DRAM through a prefetched ring) and the rgb PSUM accumulation
(bf16 identity matmul); ACT does the PSUM->SBUF T1 copy (scaled by
-wz_large); DVE does the one-op z-lerp (host pre-scales the small
slice by wz_small/wz_large) and the weight STT multiply (fp8 av-field
x per-depth -gamma*0.1*az scalar x PSUM feat).  The loop is DMA-paced
(~330 GB/s effective); normalization simplifies exactly to
(x-min)/(max-min) since EPS*(std+EPS) ~ 1e-9.
"""
import os
import sys
import numpy as np

for _p in ("/opt/trn_rl_repo", "/root/.axon_site/_ro/trn_rl_repo"):
    if os.path.isdir(_p) and _p not in sys.path:
        sys.path.insert(0, _p)

IMG = 256
NPTS = 320
MIN_D, MAX_D = 2.0, 6.0
FOCAL = 2.0
DENSITY = 0.1
EPS = 1e-8
N_CORES = 8


# ----------------------------------------------------------------------------
# host-side geometry
# ----------------------------------------------------------------------------

def _geometry(T):
    """Per-depth separable sampling params (f64). Requires R=I and Tx==Ty."""
    Tx, Ty, Tz = float(T[0]), float(T[1]), float(T[2])
    vox = 3.0 / 256.0
    half = vox * 255.0 * 0.5
    depths = np.linspace(MIN_D, MAX_D, NPTS)
    c = depths * 127.5 / (2.0 * half)
    s = c * (2.0 / 255.0)
    a = 127.5 - c - Tx * 127.5 / half
    iz = 127.5 * ((depths - Tz) / half + 1.0)
    z0 = np.floor(iz).astype(np.int64)
    fz = iz - z0
    z1 = z0 + 1
    wz0 = np.where((z0 >= 0) & (z0 < 256), 1.0 - fz, 0.0)
    wz1 = np.where((z1 >= 0) & (z1 < 256), fz, 0.0)
    az = wz0 + wz1
    q = np.arange(IMG)
    ic = a[:, None] + s[:, None] * q[None, :]
    c0 = np.floor(ic)
    fc = ic - c0
    av = (np.where((c0 >= 0) & (c0 < 256), 1.0 - fc, 0.0)
          + np.where((c0 + 1 >= 0) & (c0 + 1 < 256), fc, 0.0))
    return dict(s=s, a=a, z0=z0, z1=z1, wz0=wz0, wz1=wz1, az=az, av=av,
                active=az > 0)


def _host_inputs(vol, T):
    """Build the 8 per-core input maps. vol: (256,256,256) f32 (z,y,x)."""
    import ml_dtypes
    g = _geometry(T)
    act = np.nonzero(g["active"])[0]

    # gamma_k = prod_{j<k} (1 - 0.1*az_j): global transmittance scalars
    cfac = 1.0 - DENSITY * g["az"]
    gam = np.ones(NPTS)
    gam[1:] = np.cumprod(cfac)[:-1]
    # truncate depths whose remaining transmittance is negligible
    # (gamma < 4e-5 -> contribution ~1e-4 of the image; validated 5.7e-5
    # rel err at 96 of 240 depths)
    act = np.array([p for p in act if gam[p] > 4e-5])
    nd = int(np.ceil(len(act) / N_CORES))
    # fold gamma into the (negative) sigma u-vector
    uneg_all = (-DENSITY * (gam * g["az"])[:, None] * g["av"])
    v_all = g["av"]

    vol16 = vol.astype(ml_dtypes.bfloat16)
    in_maps = []
    for cidx in range(N_CORES):
        ks = [int(act[i]) for i in range(cidx * nd, min((cidx + 1) * nd, len(act)))]

        f8 = ml_dtypes.float8_e4m3
        slices_s = np.zeros((128, nd, 512), f8)
        slices_l = np.zeros((128, nd, 512), ml_dtypes.bfloat16)
        wlp = np.zeros((128, nd), np.float32)
        usc = np.zeros((128, nd), np.float32)
        vbs = np.zeros((128, nd, 512), f8)
        tents = np.zeros((128, nd, 512), ml_dtypes.bfloat16)
        qrow = np.arange(IMG, dtype=np.float64)
        vgrid = np.arange(256, dtype=np.float64)

        for j, p in enumerate(ks):
            w0, w1 = g["wz0"][p], g["wz1"][p]
            zz0 = min(max(int(g["z0"][p]), 0), 255)
            zz1 = min(max(int(g["z1"][p]), 0), 255)
            if w0 <= w1:
                z_small, z_large, w_small, w_large = zz0, zz1, w0, w1
            else:
                z_small, z_large, w_small, w_large = zz1, zz0, w1, w0
            # small slot: (w_small/w_large)-prescaled, fp8 (carries only
            # ~25% of the z-lerp mass); large slot: bf16
            r = np.float32(w_small / w_large)
            sl = (vol16[z_small].astype(np.float32) * r).astype(f8)
            slices_s[:, j, :] = \
                sl.reshape(2, 128, 256).transpose(1, 0, 2).reshape(128, 512)
            sl = vol16[z_large]
            slices_l[:, j, :] = \
                sl.reshape(2, 128, 256).transpose(1, 0, 2).reshape(128, 512)
            wlp[:, j] = np.float32(-w_large)
            # tent matrix A[v, q] = relu(1 - |v - ic(q)|), v = 128b + part
            ic = g["a"][p] + g["s"][p] * qrow
            A = np.clip(1.0 - np.abs(vgrid[:, None] - ic[None, :]), 0.0, None)
            # normalized sigma field av(x)av(y): exactly 1.0 in the
            # interior (fp8-exact); the -gamma*0.1*az scale rides in usc
            usc[:, j] = np.float32(-DENSITY * gam[p] * g["az"][p])
            for b in (0, 1):
                tents[:, j, 256 * b:256 * (b + 1)] = A[128 * b:128 * (b + 1)]
                vbs[:, j, 256 * b:256 * (b + 1)] = np.outer(
                    v_all[p][128 * b:128 * (b + 1)], v_all[p])

        in_maps.append({
            "slices_s": slices_s.reshape(128, nd * 512),
            "slices_l": slices_l.reshape(128, nd * 512),
            "wlp": wlp, "usc": usc,
            "vbs": vbs.reshape(128, nd * 512),
            "tents": tents.reshape(128, nd * 512),
            "identh": np.eye(128, dtype=ml_dtypes.bfloat16),
        })
    return in_maps, nd


# ----------------------------------------------------------------------------
# device program
# ----------------------------------------------------------------------------

_NC_CACHE = {}


def _build_nc(nd, sim=False):
    """sim=True replaces the AllReduce with a local DMA copy so the
    single-core TimelineSim cost model can run the program."""
    import concourse.bass as bass
    import concourse.tile as tile
    from concourse import bacc, mybir
    from contextlib import ExitStack

    dt = mybir.dt.float32
    dr = mybir.dt.float32r
    dh = mybir.dt.bfloat16
    d8 = mybir.dt.float8e4
    dhalf = mybir.dt.float16
    AF = mybir.ActivationFunctionType
    ALU = mybir.AluOpType
    AX = mybir.AxisListType.X

    nc = bacc.Bacc(None, num_devices=N_CORES)
    slcs_d = nc.dram_tensor("slices_s", [128, nd * 512], d8, kind="ExternalInput")
    slcl_d = nc.dram_tensor("slices_l", [128, nd * 512], dh, kind="ExternalInput")
    wlp_d = nc.dram_tensor("wlp", [128, nd], dt, kind="ExternalInput")
    usc_d = nc.dram_tensor("usc", [128, nd], dt, kind="ExternalInput")
    vbs_d = nc.dram_tensor("vbs", [128, nd * 512], d8, kind="ExternalInput")
    tents_d = nc.dram_tensor("tents", [128, nd * 512], dh, kind="ExternalInput")
    idh_d = nc.dram_tensor("identh", [128, 128], dh, kind="ExternalInput")
    out_d = nc.dram_tensor("out", [256, 256], dhalf, kind="ExternalOutput")


    with tile.TileContext(nc) as tc, ExitStack() as ctx:
        const = ctx.enter_context(tc.tile_pool(name="const", bufs=1))
        slp = ctx.enter_context(tc.tile_pool(name="slp", bufs=4))
        work = ctx.enter_context(tc.tile_pool(name="work", bufs=3))
        epil = ctx.enter_context(tc.tile_pool(name="epil", bufs=1))
        psum = ctx.enter_context(
            tc.tile_pool(name="psum", bufs=2, space=bass.MemorySpace.PSUM))
        pst1 = ctx.enter_context(
            tc.tile_pool(name="pst1", bufs=3, space=bass.MemorySpace.PSUM))
        psacc = ctx.enter_context(
            tc.tile_pool(name="psacc", bufs=1, space=bass.MemorySpace.PSUM))

        def cload(dram, shape, dtype=dt):
            t = const.tile(shape, dtype, tag=dram.name)
            nc.sync.dma_start(t[:], dram[:])
            return t

        wlp = cload(wlp_d, [128, nd])
        usc = cload(usc_d, [128, nd])
        identh = cload(idh_d, [128, 128], dh)

        NCH = (nd + 1) // 2
        PREF = 3
        slabs = [None] * NCH
        vbsl = [None] * NCH
        tentl = [None] * NCH

        def issue_chunk(j):
            ndep = min(2, nd - 2 * j)
            ts = slp.tile([128, ndep * 512], d8, tag="slabs")
            nc.sync.dma_start(ts[:], slcs_d[:, j * 1024:j * 1024 + ts.shape[1]])
            tl = slp.tile([128, ndep * 512], dh, tag="slabl")
            nc.sync.dma_start(tl[:], slcl_d[:, j * 1024:j * 1024 + tl.shape[1]])
            slabs[j] = (ts, tl)
            v = slp.tile([128, ndep * 512], d8, tag="vbs")
            nc.sync.dma_start(v[:], vbs_d[:, j * 1024:j * 1024 + v.shape[1]])
            vbsl[j] = v
            a = slp.tile([128, ndep * 512], dh, tag="tent")
            nc.sync.dma_start(a[:], tents_d[:, j * 1024:j * 1024 + a.shape[1]])
            tentl[j] = a

        for j in range(min(PREF, NCH)):
            issue_chunk(j)

        rgbps = psacc.tile([128, 512], dt, tag="rgb")

        # software-pipelined state
        zm_t = [None] * nd      # z-merged slice tiles
        wf_t = [None] * nd      # weighted feature tiles

        def tent_ap(k):
            return tentl[k // 2][:, (k % 2) * 512:(k % 2) * 512 + 512]

        def emit_zm(k):
            j = k // 2
            base = (k % 2) * 512
            ts, tl = slabs[j]
            zm = work.tile([128, 512], dh, tag="zm")
            nc.vector.tensor_add(zm[:], ts[:, base:base + 512],
                                 tl[:, base:base + 512])
            zm_t[k] = zm

        # prologue for depth 0
        emit_zm(0)

        for k in range(nd):
            zm = zm_t[k]
            at = tent_ap(k)

            # prefetch the slab chunk PREF ahead (once per chunk)
            if k % 2 == 0 and k // 2 + PREF < NCH:
                issue_chunk(k // 2 + PREF)

            # --- mm1: T1[x,py] = sum_y Zp[y,x] * A[y,py] ---
            t1ps = pst1.tile([128, 512], dt, tag="t1")
            for xc in (0, 1):
                for yb in (0, 1):
                    nc.tensor.matmul(
                        t1ps[:, 256 * xc:256 * (xc + 1)],
                        zm[:, 256 * yb + 128 * xc:256 * yb + 128 * xc + 128],
                        at[:, 256 * yb:256 * (yb + 1)],
                        start=(yb == 0), stop=(yb == 1))

            # PE filler while ACT does the t1 copy: prev depth's rgb acc
            if k > 0:
                nc.tensor.matmul(rgbps[:], identh[:], wf_t[k - 1][:],
                                 start=(k == 1), stop=False, skip_group_check=True)

            # --- ACT: t1sb = -wz_large * T1  (PSUM->SBUF, bf16) ---
            t1sb = work.tile([128, 512], dh, tag="t1sb")
            nc.scalar.activation(t1sb[:], t1ps[:], AF.Copy, scale=wlp[:, k:k + 1])

            # --- DVE: z-merge for next depth ---
            if k + 1 < nd:
                emit_zm(k + 1)

            # --- mm2: -feat[px,py] = sum_x A[x,px] * t1sb[x,py] ---
            featps = psum.tile([128, 512], dt, tag="feat")
            for mb in (0, 1):
                for xb in (0, 1):
                    nc.tensor.matmul(
                        featps[:, 256 * mb:256 * (mb + 1)],
                        at[:, 256 * xb + 128 * mb:256 * xb + 128 * mb + 128],
                        t1sb[:, 256 * xb:256 * (xb + 1)],
                        start=(xb == 0), stop=(xb == 1))

            # --- DVE: wf = (-gamma*sigma) .* (-feat) = gamma*sigma*feat ---
            j = k // 2
            vbk = vbsl[j][:, (k % 2) * 512:(k % 2) * 512 + 512]
            wf = work.tile([128, 512], dh, tag="wf")
            nc.vector.scalar_tensor_tensor(
                wf[:], vbk, usc[:, k:k + 1], featps[:], ALU.mult, ALU.mult)
            wf_t[k] = wf

        nc.tensor.matmul(rgbps[:], identh[:], wf_t[nd - 1][:],
                         start=False, stop=True, skip_group_check=True)

        # ---- write the per-core partial; the host sums the 8 partials
        # and applies the (x-min)/(max-min) normalization as part of the
        # gather/unshard step ----
        outsb = epil.tile([128, 512], dhalf, tag="outsb")
        nc.vector.tensor_copy(outsb[:], rgbps[:])
        nc.sync.dma_start(out_d[:].rearrange("(b p) y -> p b y", p=128),
                          outsb[:].rearrange("p (b y) -> p b y", b=2))
    return nc


# ----------------------------------------------------------------------------
# entry points
# ----------------------------------------------------------------------------

def _axis_aligned(R, T):
    return (np.allclose(np.asarray(R[0]), np.eye(3), atol=1e-6)
            and abs(float(T[0][0]) - float(T[0][1])) < 1e-12)


class _CachedSpmd:
    """Compile the PJRT executable once; repeat calls only transfer + exec."""

    def __init__(self, nc, n_cores):
        import jax
        from concourse import mybir
        from concourse.bass2jax import (_bass_exec_p, install_neuronx_cc_hook,
                                        partition_id_tensor)
        from jax.experimental.shard_map import shard_map
        from jax.sharding import Mesh, PartitionSpec
        install_neuronx_cc_hook()
        self.jax = jax
        self.n_cores = n_cores
        pname = nc.partition_id_tensor.name if nc.partition_id_tensor else None
        in_names, out_names, out_avals, zero_outs = [], [], [], []
        for alloc in nc.m.functions[0].allocations:
            if not isinstance(alloc, mybir.MemoryLocationSet):
                continue
            name = alloc.memorylocations[0].name
            if alloc.kind == "ExternalInput":
                if name != pname:
                    in_names.append(name)
            elif alloc.kind == "ExternalOutput":
                shape = tuple(alloc.tensor_shape)
                dtype = mybir.dt.np(alloc.dtype)
                out_names.append(name)
                out_avals.append(jax.core.ShapedArray(shape, dtype))
                zero_outs.append(np.zeros(shape, dtype))
        self.in_names, self.out_names = in_names, out_names
        self.out_avals, self.zero_outs = out_avals, zero_outs
        n_params, n_outs = len(in_names), len(out_names)
        all_in = list(in_names) + list(out_names)
        if pname is not None:
            all_in.append(pname)

        def _body(*args):
            operands = list(args)
            if pname is not None:
                operands.append(partition_id_tensor())
            outs = _bass_exec_p.bind(
                *operands, out_avals=tuple(out_avals), in_names=tuple(all_in),
                out_names=tuple(out_names), lowering_input_output_aliases=(),
                sim_require_finite=True, sim_require_nnan=True, nc=nc)
            return tuple(outs)

        devices = jax.devices()[:n_cores]
        mesh = Mesh(np.asarray(devices), ("core",))
        in_specs = (PartitionSpec("core"),) * (n_params + n_outs)
        out_specs = (PartitionSpec("core"),) * n_outs
        self.fn = jax.jit(shard_map(_body, mesh=mesh, in_specs=in_specs,
                                    out_specs=out_specs, check_rep=False),
                          keep_unused=True)
        self._dev_zeros = [jax.device_put(np.zeros(
            (n_cores * z.shape[0], *z.shape[1:]), z.dtype)) for z in zero_outs]

    def run(self, in_maps):
        jax = self.jax
        concat = [np.concatenate([np.asarray(in_maps[c][nm])
                                  for c in range(self.n_cores)], axis=0)
                  for nm in self.in_names]
        outs = self.fn(*concat, *self._dev_zeros)
        jax.block_until_ready(outs)
        return [{nm: np.asarray(outs[i]).reshape(
                    self.n_cores, *self.out_avals[i].shape)[c]
                 for i, nm in enumerate(self.out_names)}
                for c in range(self.n_cores)]


_RUNNER_CACHE = {}


def _run(image3d, R, T, trace=False):
    vol = np.ascontiguousarray(np.asarray(image3d, np.float32)[0, 0])
    in_maps, nd = _host_inputs(vol, np.asarray(T, np.float64)[0])
    if nd not in _NC_CACHE:
        nc = _build_nc(nd)
        nc.finalize()
        _NC_CACHE[nd] = nc
    nc = _NC_CACHE[nd]
    if id(nc) not in _RUNNER_CACHE:
        _RUNNER_CACHE[id(nc)] = _CachedSpmd(nc, N_CORES)
    results = _RUNNER_CACHE[id(nc)].run(in_maps)
    # unshard: the depth-sharded partials sum to the full image
    acc = np.zeros((256, 256), np.float64)
    for c in range(N_CORES):
        acc += np.asarray(results[c]["out"], np.float32)
    # normalization (exact reference formula; c = EPS*(std+EPS))
    s = (acc - acc.mean()) / (np.std(acc, ddof=1) + EPS)
    out = ((s - s.min() + EPS) / (s.max() - s.min() + EPS)).astype(np.float32)
    return out[None, None], results


def _numpy_fallback(image3d, R, T):
    """Direct port of the reference for non-axis-aligned cameras."""
    image3d = np.asarray(image3d, np.float32)
    R = np.asarray(R, np.float32); T = np.asarray(T, np.float32)
    B, C, D, H, W = image3d.shape
    vol = image3d[:, 0]
    vox = 3.0 / max(C, D)
    yg, xg = np.meshgrid(np.linspace(-1, 1, IMG), np.linspace(-1, 1, IMG),
                         indexing='ij')
    depths = np.linspace(MIN_D, MAX_D, NPTS)
    pcam = np.stack([xg[..., None] * depths / FOCAL,
                     yg[..., None] * depths / FOCAL,
                     np.broadcast_to(depths, (IMG, IMG, NPTS))], -1)
    v = pcam[None] - T[:, None, None, None, :]
    pw = np.einsum('bhwpj,bkj->bhwpk', v, R)
    half = np.array([vox * (W - 1) / 2, vox * (H - 1) / 2, vox * (D - 1) / 2])
    local = pw / half

    def tri(voln, pts):
        ix = (pts[..., 0] + 1) * .5 * (W - 1)
        iy = (pts[..., 1] + 1) * .5 * (H - 1)
        iz = (pts[..., 2] + 1) * .5 * (D - 1)
        out = np.zeros(ix.shape, np.float32)
        x0, y0, z0 = np.floor(ix), np.floor(iy), np.floor(iz)
        fx, fy, fz = ix - x0, iy - y0, iz - z0
        for zi, wz in ((z0, 1 - fz), (z0 + 1, fz)):
            for yi, wy in ((y0, 1 - fy), (y0 + 1, fy)):
                for xi, wx in ((x0, 1 - fx), (x0 + 1, fx)):
                    valid = ((xi >= 0) & (xi < W) & (yi >= 0) & (yi < H)
                             & (zi >= 0) & (zi < D))
                    vv = voln[np.clip(zi, 0, D - 1).astype(int),
                              np.clip(yi, 0, H - 1).astype(int),
                              np.clip(xi, 0, W - 1).astype(int)]
                    out += np.where(valid, vv * (wz * wy * wx), 0).astype(np.float32)
        return out

    feat = np.stack([tri(vol[b], local[b]) for b in range(B)])
    sigma = DENSITY * np.stack([tri(np.ones((D, H, W), np.float32), local[b])
                                for b in range(B)])
    t = (1.0 + 1e-10) - sigma
    ab = np.cumprod(t, -1)
    ab = np.concatenate([np.ones_like(ab[..., :1]), ab[..., :-1]], -1)
    rgb = np.sum(sigma * ab * feat, -1)
    out = np.transpose(rgb, (0, 2, 1))[:, None]
    s = (out - out.mean()) / (np.std(out, ddof=1) + EPS)
    return ((s - s.min() + EPS) / (s.max() - s.min() + EPS)).astype(np.float32)


def kernel(image3d, R, T):
    if not _axis_aligned(R, T):
        return _numpy_fallback(image3d, R, T)
    out, _ = _run(image3d, R, T, trace=False)
    return out



# revision 11
# speedup vs baseline: 1.4987x; 1.4987x over previous
"""Trainium2 Bass kernel for DirectVolumeRenderer (axis-aligned camera).

Factorization (per depth p, camera R=I so sample coords are separable):
    trilinear(vol) = z-lerp of 2 slices -> two matmuls with the SAME tent
    matrix  A_p[v,q] = relu(1 - |v - (a_p + s_p*q)|):
        T1   = Zp^T @ A_p          (contract y)
        feat = A_p^T @ T1          (contract x) -> image in [px,py] layout
Transmittance Gamma_k is data-independent (density is the constant 0.1,
geometry fixed): on sigma_k's support every earlier sigma_j was fully
inside its own valid square, so Gamma_k == gamma_k =
prod_{j<k}(1 - 0.1*az_j), a host scalar.  Host-side folding makes the
device program pure fp8 DoubleRow matmuls:

  zma32_k = 32 * q8( (0.1*gamma_k*az_k/S_c) * (wz0*vol[z0]+wz1*vol[z1]) )
      -- z-lerp merged on host, compositing scalar folded in, S_c = the
         core's max scalar (host multiplies the partial back by S_c);
         the x32 is exact in fp8 and pre-scales T1 for the cast below
  zmb_k   = q8( 32*(zm_k - q8(zm_k)) )       -- fp8 residual stream
  At_k    = q8( av_k(q) * relu(1 - |v - ic_k(q)|) ), ic snapped to 1/16
      -- (1-f, f) pairs exact in e4m3; av folded in covers the sigma
         field, so the compositing weight is a pure scalar

Per depth (all matmuls fp8e4 DoubleRow, 256-deep contraction):
  mm1:  psS  = zma32^T At + zmb^T At  =  32*T1_true   (one bank)  [PE]
  t1sbA = q8(psS * 1/32)              (scaled cast of true T1)   [ACT]
  t1sbB = q8((t1sbA * -32) + psS)     (total residual, x32, fp8) [DVE]
  rgb  += At^T t1sbA ; rgb2 += At^T t1sbB    (PSUM-accumulated)   [PE]
Tail: rgb += (I/32) @ bf16(rgb2) closes the accumulation, fp16 out DMA.
The residual stream cancels both the zm and the T1-cast fp8 noise
(which the flat image's (x-min)/(max-min) normalization amplifies ~6x);
remaining error is the 1/16 tent snap, ~1e-2 total vs the 2e-2 gate.
Per-core HBM traffic: nd*192KB + 160KB."""
import os
import sys
import numpy as np

for _p in ("/opt/trn_rl_repo", "/root/.axon_site/_ro/trn_rl_repo"):
    if os.path.isdir(_p) and _p not in sys.path:
        sys.path.insert(0, _p)

IMG = 256
NPTS = 320
MIN_D, MAX_D = 2.0, 6.0
FOCAL = 2.0
DENSITY = 0.1
EPS = 1e-8
N_CORES = 8
ND = 8           # depths per core (8*ND depths kept in front-to-back order)


# ----------------------------------------------------------------------------
# host-side geometry
# ----------------------------------------------------------------------------

def _geometry(T):
    """Per-depth separable sampling params (f64). Requires R=I and Tx==Ty."""
    Tx, Ty, Tz = float(T[0]), float(T[1]), float(T[2])
    vox = 3.0 / 256.0
    half = vox * 255.0 * 0.5
    depths = np.linspace(MIN_D, MAX_D, NPTS)
    c = depths * 127.5 / (2.0 * half)
    s = c * (2.0 / 255.0)
    a = 127.5 - c - Tx * 127.5 / half
    iz = 127.5 * ((depths - Tz) / half + 1.0)
    z0 = np.floor(iz).astype(np.int64)
    fz = iz - z0
    z1 = z0 + 1
    wz0 = np.where((z0 >= 0) & (z0 < 256), 1.0 - fz, 0.0)
    wz1 = np.where((z1 >= 0) & (z1 < 256), fz, 0.0)
    az = wz0 + wz1
    return dict(s=s, a=a, z0=z0, z1=z1, wz0=wz0, wz1=wz1, az=az, active=az > 0)


def _blk(m):
    """(256, N) f32 -> (128, 2*N) with row p = [t0 block | t1 block]."""
    n = m.shape[1]
    return np.ascontiguousarray(
        m.reshape(2, 128, n).transpose(1, 0, 2).reshape(128, 2 * n))


def _host_inputs(vol, T):
    """Build the 8 per-core input maps + per-core output scales.

    vol: (256,256,256) f32 (z,y,x)."""
    import ml_dtypes
    f8 = ml_dtypes.float8_e4m3
    g = _geometry(T)
    act = np.nonzero(g["active"])[0]

    # gamma_k = prod_{j<k} (1 - 0.1*az_j): global transmittance scalars
    cfac = 1.0 - DENSITY * g["az"]
    gam = np.ones(NPTS)
    gam[1:] = np.cumprod(cfac)[:-1]

    nk = min(N_CORES * ND, len(act))
    act = act[:nk]
    nd = ND
    wk = DENSITY * gam * g["az"]          # per-depth compositing scalar

    qrow = np.arange(IMG, dtype=np.float64)
    vgrid = np.arange(256, dtype=np.float64)
    identh32 = (np.eye(128, dtype=np.float32) / 32.0).astype(ml_dtypes.bfloat16)

    in_maps, s_cores = [], []
    for cidx in range(N_CORES):
        ks = [int(act[i]) for i in range(cidx * nd, min((cidx + 1) * nd, nk))]
        s_c = max(float(wk[p]) for p in ks) if ks else 1.0
        stream = np.zeros((128, nd * 1536), f8)
        for j, p in enumerate(ks):
            zz0 = min(max(int(g["z0"][p]), 0), 255)
            zz1 = min(max(int(g["z1"][p]), 0), 255)
            zm = (g["wz0"][p] * vol[zz0].astype(np.float64)
                  + g["wz1"][p] * vol[zz1].astype(np.float64))
            zm = zm * (wk[p] / s_c)
            zma = zm.astype(np.float32).astype(f8)
            zma32 = (zma.astype(np.float32) * 32.0)
            zmb = ((zm - zma.astype(np.float64)) * 32.0).astype(np.float32)
            # tent A[v, q] on a 1/16-snapped grid so (1-f, f) pairs are
            # exact in e4m3; av (validity weight sum) folded into columns
            ic = np.round((g["a"][p] + g["s"][p] * qrow) * 16.0) / 16.0
            c0 = np.floor(ic)
            fc = ic - c0
            av = (np.where((c0 >= 0) & (c0 < 256), 1.0 - fc, 0.0)
                  + np.where((c0 + 1 >= 0) & (c0 + 1 < 256), fc, 0.0))
            A = np.clip(1.0 - np.abs(vgrid[:, None] - ic[None, :]), 0.0, None)
            base = j * 1536
            stream[:, base:base + 512] = _blk(zma32)
            stream[:, base + 512:base + 1024] = _blk(zmb)
            stream[:, base + 1024:base + 1536] = _blk(
                (A * av[None, :]).astype(np.float32))
        in_maps.append({"stream": stream, "identh32": identh32})
        s_cores.append(s_c)
    return in_maps, nd, s_cores


# ----------------------------------------------------------------------------
# device program
# ----------------------------------------------------------------------------

_NC_CACHE = {}


def _build_nc(nd):
    import concourse.bass as bass
    import concourse.tile as tile
    from concourse import bacc, mybir
    from contextlib import ExitStack

    dt = mybir.dt.float32
    d8 = mybir.dt.float8e4
    dh = mybir.dt.bfloat16
    dhalf = mybir.dt.float16
    AF = mybir.ActivationFunctionType
    ALU = mybir.AluOpType
    DR = mybir.MatmulPerfMode.DoubleRow

    nc = bacc.Bacc(None, num_devices=N_CORES)
    str_d = nc.dram_tensor("stream", [128, nd * 1536], d8, kind="ExternalInput")
    idh_d = nc.dram_tensor("identh32", [128, 128], dh, kind="ExternalInput")
    out_d = nc.dram_tensor("out", [256, 256], dhalf, kind="ExternalOutput")

    CH = 2                       # depths per DMA chunk
    NCH = (nd + CH - 1) // CH
    PREF = 3

    with tile.TileContext(nc) as tc, ExitStack() as ctx:
        const = ctx.enter_context(tc.tile_pool(name="const", bufs=1))
        slp = ctx.enter_context(tc.tile_pool(name="slp", bufs=PREF + 1))
        work = ctx.enter_context(tc.tile_pool(name="work", bufs=3))
        epil = ctx.enter_context(tc.tile_pool(name="epil", bufs=1))
        psS = ctx.enter_context(
            tc.tile_pool(name="psS", bufs=4, space=bass.MemorySpace.PSUM))
        psacc = ctx.enter_context(
            tc.tile_pool(name="psacc", bufs=1, space=bass.MemorySpace.PSUM))

        identh32 = const.tile([128, 128], dh, tag="identh32")
        nc.sync.dma_start(identh32[:], idh_d[:])

        chunks = [None] * NCH

        def issue_chunk(j):
            ndep = min(CH, nd - CH * j)
            t = slp.tile([128, ndep * 6, 256], d8, tag="chunk")
            nc.sync.dma_start(
                t[:].rearrange("p t x -> p (t x)"),
                str_d[:, j * CH * 1536:j * CH * 1536 + ndep * 1536])
            chunks[j] = t

        for j in range(min(PREF, NCH)):
            issue_chunk(j)

        rgbps = psacc.tile([128, 512], dt, tag="rgb")
        rgb2ps = psacc.tile([128, 512], dt, tag="rgb2")
        t1S = [None] * nd

        def views(k):
            o = (k % CH) * 6
            t = chunks[k // CH]
            return (t[:, o:o + 2, :], t[:, o + 2:o + 4, :],
                    t[:, o + 4:o + 6, :])

        def emit_mm1(k):
            # psS[x, py] = 32*T1_true = sum_y (zma32 + zmb)[y, x] At[y, py];
            # DoubleRow contracts y = t*128 + part in one instruction
            zma, zmb, at = views(k)
            ps = psS.tile([128, 512], dt, tag="t1s")
            # start=True marks the WHOLE 2KB psum bank pending-zero, so
            # only the bank's first matmul may set it; later first-writes
            # to other byte ranges land on pending bytes (= write-fresh)
            for b in (0, 1):
                nc.tensor.matmul(ps[:, 256 * b:256 * (b + 1)],
                                 zma[:, :, 128 * b:128 * (b + 1)], at[:],
                                 start=(b == 0), stop=False, perf_mode=DR)
                nc.tensor.matmul(ps[:, 256 * b:256 * (b + 1)],
                                 zmb[:, :, 128 * b:128 * (b + 1)], at[:],
                                 start=False, stop=(b == 1), perf_mode=DR)
            t1S[k] = ps

        emit_mm1(0)

        for k in range(nd):
            if k % CH == 0 and k // CH + PREF < NCH:
                issue_chunk(k // CH + PREF)

            ps = t1S[k]
            # --- ACT: t1sbA = q8(T1_true) via scaled cast ---
            sbA = work.tile([128, 2, 256], d8, tag="t1sbA")
            sbAf = sbA[:].rearrange("p t x -> p (t x)")
            nc.scalar.activation(sbAf, ps[:], AF.Copy, scale=1.0 / 32.0)
            # --- DVE: t1sbB = q8(psS - 32*t1sbA)  (total residual, x32) ---
            sbB = work.tile([128, 2, 256], d8, tag="t1sbB")
            nc.vector.scalar_tensor_tensor(
                sbB[:].rearrange("p t x -> p (t x)"), sbAf, -32.0, ps[:],
                ALU.mult, ALU.add)

            # --- keep PE ahead: mm1 for the next depth first ---
            if k + 1 < nd:
                emit_mm1(k + 1)

            # --- mm2: rgb += At^T T1a ; rgb2 += At^T t1sbB ---
            _, _, at = views(k)
            for m in (0, 1):
                nc.tensor.matmul(rgbps[:, 256 * m:256 * (m + 1)],
                                 at[:, :, 128 * m:128 * (m + 1)], sbA[:],
                                 start=(k == 0 and m == 0), stop=False,
                                 perf_mode=DR, skip_group_check=True)
            for m in (0, 1):
                nc.tensor.matmul(rgb2ps[:, 256 * m:256 * (m + 1)],
                                 at[:, :, 128 * m:128 * (m + 1)], sbB[:],
                                 start=(k == 0 and m == 0),
                                 stop=(k == nd - 1 and m == 1),
                                 perf_mode=DR, skip_group_check=True)

        # ---- tail: rgb += (I/32) @ bf16(rgb2); cast + store fp16 ----
        rgb2sb = epil.tile([128, 512], dh, tag="rgb2sb")
        nc.scalar.activation(rgb2sb[:, 0:256], rgb2ps[:, 0:256], AF.Copy)
        nc.vector.tensor_copy(rgb2sb[:, 256:512], rgb2ps[:, 256:512])
        nc.tensor.matmul(rgbps[:], identh32[:], rgb2sb[:],
                         start=False, stop=True, skip_group_check=True)
        outsb = epil.tile([128, 512], dhalf, tag="outsb")
        nc.scalar.activation(outsb[:, 0:256], rgbps[:, 0:256], AF.Copy)
        nc.vector.tensor_copy(outsb[:, 256:512], rgbps[:, 256:512])
        nc.sync.dma_start(out_d[:].rearrange("(b p) y -> p b y", p=128),
                          outsb[:].rearrange("p (b y) -> p b y", b=2))
    return nc


# ----------------------------------------------------------------------------
# entry points
# ----------------------------------------------------------------------------

def _axis_aligned(R, T):
    return (np.allclose(np.asarray(R[0]), np.eye(3), atol=1e-6)
            and abs(float(T[0][0]) - float(T[0][1])) < 1e-12)


class _CachedSpmd:
    """Compile the PJRT executable once; repeat calls only transfer + exec."""

    def __init__(self, nc, n_cores):
        import jax
        from concourse import mybir
        from concourse.bass2jax import (_bass_exec_p, install_neuronx_cc_hook,
                                        partition_id_tensor)
        from jax.experimental.shard_map import shard_map
        from jax.sharding import Mesh, PartitionSpec
        install_neuronx_cc_hook()
        self.jax = jax
        self.n_cores = n_cores
        pname = nc.partition_id_tensor.name if nc.partition_id_tensor else None
        in_names, out_names, out_avals, zero_outs = [], [], [], []
        for alloc in nc.m.functions[0].allocations:
            if not isinstance(alloc, mybir.MemoryLocationSet):
                continue
            name = alloc.memorylocations[0].name
            if alloc.kind == "ExternalInput":
                if name != pname:
                    in_names.append(name)
            elif alloc.kind == "ExternalOutput":
                shape = tuple(alloc.tensor_shape)
                dtype = mybir.dt.np(alloc.dtype)
                out_names.append(name)
                out_avals.append(jax.core.ShapedArray(shape, dtype))
                zero_outs.append(np.zeros(shape, dtype))
        self.in_names, self.out_names = in_names, out_names
        self.out_avals, self.zero_outs = out_avals, zero_outs
        n_params, n_outs = len(in_names), len(out_names)
        all_in = list(in_names) + list(out_names)
        if pname is not None:
            all_in.append(pname)

        def _body(*args):
            operands = list(args)
            if pname is not None:
                operands.append(partition_id_tensor())
            outs = _bass_exec_p.bind(
                *operands, out_avals=tuple(out_avals), in_names=tuple(all_in),
                out_names=tuple(out_names), lowering_input_output_aliases=(),
                sim_require_finite=True, sim_require_nnan=True, nc=nc)
            return tuple(outs)

        devices = jax.devices()[:n_cores]
        mesh = Mesh(np.asarray(devices), ("core",))
        in_specs = (PartitionSpec("core"),) * (n_params + n_outs)
        out_specs = (PartitionSpec("core"),) * n_outs
        self.fn = jax.jit(shard_map(_body, mesh=mesh, in_specs=in_specs,
                                    out_specs=out_specs, check_rep=False),
                          keep_unused=True)
        self._dev_zeros = [jax.device_put(np.zeros(
            (n_cores * z.shape[0], *z.shape[1:]), z.dtype)) for z in zero_outs]

    def run(self, in_maps):
        jax = self.jax
        concat = [np.concatenate([np.asarray(in_maps[c][nm])
                                  for c in range(self.n_cores)], axis=0)
                  for nm in self.in_names]
        outs = self.fn(*concat, *self._dev_zeros)
        jax.block_until_ready(outs)
        return [{nm: np.asarray(outs[i]).reshape(
                    self.n_cores, *self.out_avals[i].shape)[c]
                 for i, nm in enumerate(self.out_names)}
                for c in range(self.n_cores)]


_RUNNER_CACHE = {}


def _run(image3d, R, T, trace=False):
    vol = np.ascontiguousarray(np.asarray(image3d, np.float32)[0, 0])
    in_maps, nd, s_cores = _host_inputs(vol, np.asarray(T, np.float64)[0])
    if nd not in _NC_CACHE:
        nc = _build_nc(nd)
        nc.finalize()
        _NC_CACHE[nd] = nc
    nc = _NC_CACHE[nd]
    if id(nc) not in _RUNNER_CACHE:
        _RUNNER_CACHE[id(nc)] = _CachedSpmd(nc, N_CORES)
    results = _RUNNER_CACHE[id(nc)].run(in_maps)
    # unshard: the S_c-scaled depth-sharded partials sum to the full image
    acc = np.zeros((256, 256), np.float64)
    for c in range(N_CORES):
        acc += s_cores[c] * np.asarray(results[c]["out"], np.float32)
    # normalization (exact reference formula)
    s = (acc - acc.mean()) / (np.std(acc, ddof=1) + EPS)
    out = ((s - s.min() + EPS) / (s.max() - s.min() + EPS)).astype(np.float32)
    return out[None, None], results


def _numpy_fallback(image3d, R, T):
    """Direct port of the reference for non-axis-aligned cameras."""
    image3d = np.asarray(image3d, np.float32)
    R = np.asarray(R, np.float32); T = np.asarray(T, np.float32)
    B, C, D, H, W = image3d.shape
    vol = image3d[:, 0]
    vox = 3.0 / max(C, D)
    yg, xg = np.meshgrid(np.linspace(-1, 1, IMG), np.linspace(-1, 1, IMG),
                         indexing='ij')
    depths = np.linspace(MIN_D, MAX_D, NPTS)
    pcam = np.stack([xg[..., None] * depths / FOCAL,
                     yg[..., None] * depths / FOCAL,
                     np.broadcast_to(depths, (IMG, IMG, NPTS))], -1)
    v = pcam[None] - T[:, None, None, None, :]
    pw = np.einsum('bhwpj,bkj->bhwpk', v, R)
    half = np.array([vox * (W - 1) / 2, vox * (H - 1) / 2, vox * (D - 1) / 2])
    local = pw / half

    def tri(voln, pts):
        ix = (pts[..., 0] + 1) * .5 * (W - 1)
        iy = (pts[..., 1] + 1) * .5 * (H - 1)
        iz = (pts[..., 2] + 1) * .5 * (D - 1)
        out = np.zeros(ix.shape, np.float32)
        x0, y0, z0 = np.floor(ix), np.floor(iy), np.floor(iz)
        fx, fy, fz = ix - x0, iy - y0, iz - z0
        for zi, wz in ((z0, 1 - fz), (z0 + 1, fz)):
            for yi, wy in ((y0, 1 - fy), (y0 + 1, fy)):
                for xi, wx in ((x0, 1 - fx), (x0 + 1, fx)):
                    valid = ((xi >= 0) & (xi < W) & (yi >= 0) & (yi < H)
                             & (zi >= 0) & (zi < D))
                    vv = voln[np.clip(zi, 0, D - 1).astype(int),
                              np.clip(yi, 0, H - 1).astype(int),
                              np.clip(xi, 0, W - 1).astype(int)]
                    out += np.where(valid, vv * (wz * wy * wx), 0).astype(np.float32)
        return out

    feat = np.stack([tri(vol[b], local[b]) for b in range(B)])
    sigma = DENSITY * np.stack([tri(np.ones((D, H, W), np.float32), local[b])
                                for b in range(B)])
    t = (1.0 + 1e-10) - sigma
    ab = np.cumprod(t, -1)
    ab = np.concatenate([np.ones_like(ab[..., :1]), ab[..., :-1]], -1)
    rgb = np.sum(sigma * ab * feat, -1)
    out = np.transpose(rgb, (0, 2, 1))[:, None]
    s = (out - out.mean()) / (np.std(out, ddof=1) + EPS)
    return ((s - s.min() + EPS) / (s.max() - s.min() + EPS)).astype(np.float32)


def kernel(image3d, R, T):
    if not _axis_aligned(R, T):
        return _numpy_fallback(image3d, R, T)
    out, _ = _run(image3d, R, T, trace=False)
    return out


# revision 12
# speedup vs baseline: 1.7536x; 1.1701x over previous
"""Trainium2 Bass kernel for DirectVolumeRenderer (axis-aligned camera).

Factorization (per depth p, camera R=I so sample coords are separable):
    trilinear(vol) = z-lerp of 2 slices -> two matmuls with the SAME tent
    matrix  A_p[v,q] = relu(1 - |v - (a_p + s_p*q)|):
        T1   = Zp^T @ A_p          (contract y)
        feat = A_p^T @ T1          (contract x) -> image in [px,py] layout
Transmittance Gamma_k is data-independent (density is the constant 0.1,
geometry fixed): on sigma_k's support every earlier sigma_j was fully
inside its own valid square, so Gamma_k == gamma_k =
prod_{j<k}(1 - 0.1*az_j), a host scalar.  Host-side folding leaves the
device a single bf16 matmul chain per depth:

  zm_k = bf16( (0.1*gamma_k*az_k/S_c) * (wz0*vol[z0] + wz1*vol[z1]) )
      -- z-lerp merged on host, compositing scalar folded in, S_c = the
         core's max scalar (host multiplies the partial back by S_c)
  At_k = bf16( av_k(q) * relu(1 - |v - ic_k(q)|) )
      -- av (validity weight sum) folded into columns covers the sigma
         field, so the compositing weight is a pure scalar

Per depth: mm1 (4 bf16 matmuls) -> psT; one ACT/DVE-split bf16 cast;
mm2 (4 bf16 matmuls) accumulated into a single rgb PSUM bank across all
depths (start only on the bank's first matmul -- start=True marks the
WHOLE 2KB bank pending-zero, so later region-first writes land on
pending bytes and write fresh).  No compositing chain, no identity
matmul, no residual streams; fp8 was tried and is NOT faster here (PE
fp8 DoubleRow doubles contraction depth, not column rate, so the
fp8+residual scheme costs 2x this PE time at worse accuracy).
Per-core HBM traffic: nd*256KB + 128KB out."""
import os
import sys
import numpy as np

for _p in ("/opt/trn_rl_repo", "/root/.axon_site/_ro/trn_rl_repo"):
    if os.path.isdir(_p) and _p not in sys.path:
        sys.path.insert(0, _p)

IMG = 256
NPTS = 320
MIN_D, MAX_D = 2.0, 6.0
FOCAL = 2.0
DENSITY = 0.1
EPS = 1e-8
N_CORES = 8
ND = 6           # depths per core (8*ND depths kept in front-to-back order)


# ----------------------------------------------------------------------------
# host-side geometry
# ----------------------------------------------------------------------------

def _geometry(T):
    """Per-depth separable sampling params (f64). Requires R=I and Tx==Ty."""
    Tx, Ty, Tz = float(T[0]), float(T[1]), float(T[2])
    vox = 3.0 / 256.0
    half = vox * 255.0 * 0.5
    depths = np.linspace(MIN_D, MAX_D, NPTS)
    c = depths * 127.5 / (2.0 * half)
    s = c * (2.0 / 255.0)
    a = 127.5 - c - Tx * 127.5 / half
    iz = 127.5 * ((depths - Tz) / half + 1.0)
    z0 = np.floor(iz).astype(np.int64)
    fz = iz - z0
    z1 = z0 + 1
    wz0 = np.where((z0 >= 0) & (z0 < 256), 1.0 - fz, 0.0)
    wz1 = np.where((z1 >= 0) & (z1 < 256), fz, 0.0)
    az = wz0 + wz1
    return dict(s=s, a=a, z0=z0, z1=z1, wz0=wz0, wz1=wz1, az=az, active=az > 0)


def _blk(m):
    """(256, N) f32 -> (128, 2*N) with row p = [t0 block | t1 block]."""
    n = m.shape[1]
    return np.ascontiguousarray(
        m.reshape(2, 128, n).transpose(1, 0, 2).reshape(128, 2 * n))


def _host_inputs(vol, T):
    """Build the 8 per-core input maps + per-core output scales.

    vol: (256,256,256) f32 (z,y,x)."""
    import ml_dtypes
    bf = ml_dtypes.bfloat16
    g = _geometry(T)
    act = np.nonzero(g["active"])[0]

    # gamma_k = prod_{j<k} (1 - 0.1*az_j): global transmittance scalars
    cfac = 1.0 - DENSITY * g["az"]
    gam = np.ones(NPTS)
    gam[1:] = np.cumprod(cfac)[:-1]

    nk = min(N_CORES * ND, len(act))
    act = act[:nk]
    nd = ND
    wk = DENSITY * gam * g["az"]          # per-depth compositing scalar

    qrow = np.arange(IMG, dtype=np.float64)
    vgrid = np.arange(256, dtype=np.float64)

    in_maps, s_cores = [], []
    for cidx in range(N_CORES):
        ks = [int(act[i]) for i in range(cidx * nd, min((cidx + 1) * nd, nk))]
        s_c = max(float(wk[p]) for p in ks) if ks else 1.0
        stream = np.zeros((128, nd * 1024), bf)
        for j, p in enumerate(ks):
            zz0 = min(max(int(g["z0"][p]), 0), 255)
            zz1 = min(max(int(g["z1"][p]), 0), 255)
            zm = (g["wz0"][p] * vol[zz0].astype(np.float64)
                  + g["wz1"][p] * vol[zz1].astype(np.float64))
            zm = (zm * (wk[p] / s_c)).astype(np.float32)
            # tent A[v, q] with av (validity weight sum) folded into columns
            ic = g["a"][p] + g["s"][p] * qrow
            c0 = np.floor(ic)
            fc = ic - c0
            av = (np.where((c0 >= 0) & (c0 < 256), 1.0 - fc, 0.0)
                  + np.where((c0 + 1 >= 0) & (c0 + 1 < 256), fc, 0.0))
            A = np.clip(1.0 - np.abs(vgrid[:, None] - ic[None, :]), 0.0, None)
            base = j * 1024
            stream[:, base:base + 512] = _blk(zm)
            stream[:, base + 512:base + 1024] = _blk(
                (A * av[None, :]).astype(np.float32))
        in_maps.append({"stream": stream})
        s_cores.append(s_c)
    return in_maps, nd, s_cores


# ----------------------------------------------------------------------------
# device program
# ----------------------------------------------------------------------------

_NC_CACHE = {}


def _build_nc(nd):
    import concourse.bass as bass
    import concourse.tile as tile
    from concourse import bacc, mybir
    from contextlib import ExitStack

    dt = mybir.dt.float32
    dh = mybir.dt.bfloat16
    dhalf = mybir.dt.float16
    AF = mybir.ActivationFunctionType

    nc = bacc.Bacc(None, num_devices=N_CORES)
    str_d = nc.dram_tensor("stream", [128, nd * 1024], dh, kind="ExternalInput")
    out_d = nc.dram_tensor("out", [256, 256], dhalf, kind="ExternalOutput")

    CH = 2                       # depths per DMA chunk
    NCH = (nd + CH - 1) // CH
    PREF = 2

    with tile.TileContext(nc) as tc, ExitStack() as ctx:
        slp = ctx.enter_context(tc.tile_pool(name="slp", bufs=PREF + 1))
        work = ctx.enter_context(tc.tile_pool(name="work", bufs=3))
        epil = ctx.enter_context(tc.tile_pool(name="epil", bufs=1))
        psT = ctx.enter_context(
            tc.tile_pool(name="psT", bufs=3, space=bass.MemorySpace.PSUM))
        psacc = ctx.enter_context(
            tc.tile_pool(name="psacc", bufs=1, space=bass.MemorySpace.PSUM))

        chunks = [None] * NCH

        def issue_chunk(j):
            ndep = min(CH, nd - CH * j)
            t = slp.tile([128, ndep * 4, 256], dh, tag="chunk")
            nc.sync.dma_start(
                t[:].rearrange("p t x -> p (t x)"),
                str_d[:, j * CH * 1024:j * CH * 1024 + ndep * 1024])
            chunks[j] = t

        for j in range(min(PREF, NCH)):
            issue_chunk(j)

        rgbps = psacc.tile([128, 512], dt, tag="rgb")
        t1S = [None] * nd

        def views(k):
            o = (k % CH) * 4
            t = chunks[k // CH]
            return t[:, o:o + 2, :], t[:, o + 2:o + 4, :]

        def emit_mm1(k):
            # T1[x, py] = sum_y zm[y, x] At[y, py], y = t*128 + part.
            # start=True marks the WHOLE 2KB psum bank pending-zero, so
            # only the bank's first matmul sets it; later first-writes to
            # other byte ranges land on pending bytes (= write fresh).
            zm, at = views(k)
            ps = psT.tile([128, 512], dt, tag="t1s")
            for b in (0, 1):
                for t in (0, 1):
                    nc.tensor.matmul(ps[:, 256 * b:256 * (b + 1)],
                                     zm[:, t, 128 * b:128 * (b + 1)],
                                     at[:, t, :],
                                     start=(b == 0 and t == 0),
                                     stop=(b == 1 and t == 1))
            t1S[k] = ps

        emit_mm1(0)

        for k in range(nd):
            if k % CH == 0 and k // CH + PREF < NCH:
                issue_chunk(k // CH + PREF)

            ps = t1S[k]
            # --- T1 cast PSUM f32 -> SBUF bf16, split ACT | DVE ---
            t1sb = work.tile([128, 2, 256], dh, tag="t1sb")
            nc.scalar.activation(t1sb[:, 0, :], ps[:, 0:256], AF.Copy)
            nc.vector.tensor_copy(t1sb[:, 1, :], ps[:, 256:512])

            # --- keep PE ahead: mm1 for the next depth first ---
            if k + 1 < nd:
                emit_mm1(k + 1)

            # --- mm2: rgb[px, py] += sum_x At[x, px] T1[x, py] ---
            _, at = views(k)
            for m in (0, 1):
                for t in (0, 1):
                    nc.tensor.matmul(rgbps[:, 256 * m:256 * (m + 1)],
                                     at[:, t, 128 * m:128 * (m + 1)],
                                     t1sb[:, t, :],
                                     start=(k == 0 and m == 0 and t == 0),
                                     stop=(k == nd - 1 and m == 1 and t == 1),
                                     skip_group_check=True)

        # ---- per-core partial out; host scales by S_c, sums, normalizes ----
        outsb = epil.tile([128, 512], dhalf, tag="outsb")
        nc.scalar.activation(outsb[:, 0:256], rgbps[:, 0:256], AF.Copy)
        nc.vector.tensor_copy(outsb[:, 256:512], rgbps[:, 256:512])
        nc.sync.dma_start(out_d[:].rearrange("(b p) y -> p b y", p=128),
                          outsb[:].rearrange("p (b y) -> p b y", b=2))
    return nc


# ----------------------------------------------------------------------------
# entry points
# ----------------------------------------------------------------------------

def _axis_aligned(R, T):
    return (np.allclose(np.asarray(R[0]), np.eye(3), atol=1e-6)
            and abs(float(T[0][0]) - float(T[0][1])) < 1e-12)


class _CachedSpmd:
    """Compile the PJRT executable once; repeat calls only transfer + exec."""

    def __init__(self, nc, n_cores):
        import jax
        from concourse import mybir
        from concourse.bass2jax import (_bass_exec_p, install_neuronx_cc_hook,
                                        partition_id_tensor)
        from jax.experimental.shard_map import shard_map
        from jax.sharding import Mesh, PartitionSpec
        install_neuronx_cc_hook()
        self.jax = jax
        self.n_cores = n_cores
        pname = nc.partition_id_tensor.name if nc.partition_id_tensor else None
        in_names, out_names, out_avals, zero_outs = [], [], [], []
        for alloc in nc.m.functions[0].allocations:
            if not isinstance(alloc, mybir.MemoryLocationSet):
                continue
            name = alloc.memorylocations[0].name
            if alloc.kind == "ExternalInput":
                if name != pname:
                    in_names.append(name)
            elif alloc.kind == "ExternalOutput":
                shape = tuple(alloc.tensor_shape)
                dtype = mybir.dt.np(alloc.dtype)
                out_names.append(name)
                out_avals.append(jax.core.ShapedArray(shape, dtype))
                zero_outs.append(np.zeros(shape, dtype))
        self.in_names, self.out_names = in_names, out_names
        self.out_avals, self.zero_outs = out_avals, zero_outs
        n_params, n_outs = len(in_names), len(out_names)
        all_in = list(in_names) + list(out_names)
        if pname is not None:
            all_in.append(pname)

        def _body(*args):
            operands = list(args)
            if pname is not None:
                operands.append(partition_id_tensor())
            outs = _bass_exec_p.bind(
                *operands, out_avals=tuple(out_avals), in_names=tuple(all_in),
                out_names=tuple(out_names), lowering_input_output_aliases=(),
                sim_require_finite=True, sim_require_nnan=True, nc=nc)
            return tuple(outs)

        devices = jax.devices()[:n_cores]
        mesh = Mesh(np.asarray(devices), ("core",))
        in_specs = (PartitionSpec("core"),) * (n_params + n_outs)
        out_specs = (PartitionSpec("core"),) * n_outs
        self.fn = jax.jit(shard_map(_body, mesh=mesh, in_specs=in_specs,
                                    out_specs=out_specs, check_rep=False),
                          keep_unused=True)
        self._dev_zeros = [jax.device_put(np.zeros(
            (n_cores * z.shape[0], *z.shape[1:]), z.dtype)) for z in zero_outs]

    def run(self, in_maps):
        jax = self.jax
        concat = [np.concatenate([np.asarray(in_maps[c][nm])
                                  for c in range(self.n_cores)], axis=0)
                  for nm in self.in_names]
        outs = self.fn(*concat, *self._dev_zeros)
        jax.block_until_ready(outs)
        return [{nm: np.asarray(outs[i]).reshape(
                    self.n_cores, *self.out_avals[i].shape)[c]
                 for i, nm in enumerate(self.out_names)}
                for c in range(self.n_cores)]


_RUNNER_CACHE = {}


def _run(image3d, R, T, trace=False):
    vol = np.ascontiguousarray(np.asarray(image3d, np.float32)[0, 0])
    in_maps, nd, s_cores = _host_inputs(vol, np.asarray(T, np.float64)[0])
    if nd not in _NC_CACHE:
        nc = _build_nc(nd)
        nc.finalize()
        _NC_CACHE[nd] = nc
    nc = _NC_CACHE[nd]
    if id(nc) not in _RUNNER_CACHE:
        _RUNNER_CACHE[id(nc)] = _CachedSpmd(nc, N_CORES)
    results = _RUNNER_CACHE[id(nc)].run(in_maps)
    # unshard: the S_c-scaled depth-sharded partials sum to the full image
    acc = np.zeros((256, 256), np.float64)
    for c in range(N_CORES):
        acc += s_cores[c] * np.asarray(results[c]["out"], np.float32)
    # normalization (exact reference formula)
    s = (acc - acc.mean()) / (np.std(acc, ddof=1) + EPS)
    out = ((s - s.min() + EPS) / (s.max() - s.min() + EPS)).astype(np.float32)
    return out[None, None], results


def _numpy_fallback(image3d, R, T):
    """Direct port of the reference for non-axis-aligned cameras."""
    image3d = np.asarray(image3d, np.float32)
    R = np.asarray(R, np.float32); T = np.asarray(T, np.float32)
    B, C, D, H, W = image3d.shape
    vol = image3d[:, 0]
    vox = 3.0 / max(C, D)
    yg, xg = np.meshgrid(np.linspace(-1, 1, IMG), np.linspace(-1, 1, IMG),
                         indexing='ij')
    depths = np.linspace(MIN_D, MAX_D, NPTS)
    pcam = np.stack([xg[..., None] * depths / FOCAL,
                     yg[..., None] * depths / FOCAL,
                     np.broadcast_to(depths, (IMG, IMG, NPTS))], -1)
    v = pcam[None] - T[:, None, None, None, :]
    pw = np.einsum('bhwpj,bkj->bhwpk', v, R)
    half = np.array([vox * (W - 1) / 2, vox * (H - 1) / 2, vox * (D - 1) / 2])
    local = pw / half

    def tri(voln, pts):
        ix = (pts[..., 0] + 1) * .5 * (W - 1)
        iy = (pts[..., 1] + 1) * .5 * (H - 1)
        iz = (pts[..., 2] + 1) * .5 * (D - 1)
        out = np.zeros(ix.shape, np.float32)
        x0, y0, z0 = np.floor(ix), np.floor(iy), np.floor(iz)
        fx, fy, fz = ix - x0, iy - y0, iz - z0
        for zi, wz in ((z0, 1 - fz), (z0 + 1, fz)):
            for yi, wy in ((y0, 1 - fy), (y0 + 1, fy)):
                for xi, wx in ((x0, 1 - fx), (x0 + 1, fx)):
                    valid = ((xi >= 0) & (xi < W) & (yi >= 0) & (yi < H)
                             & (zi >= 0) & (zi < D))
                    vv = voln[np.clip(zi, 0, D - 1).astype(int),
                              np.clip(yi, 0, H - 1).astype(int),
                              np.clip(xi, 0, W - 1).astype(int)]
                    out += np.where(valid, vv * (wz * wy * wx), 0).astype(np.float32)
        return out

    feat = np.stack([tri(vol[b], local[b]) for b in range(B)])
    sigma = DENSITY * np.stack([tri(np.ones((D, H, W), np.float32), local[b])
                                for b in range(B)])
    t = (1.0 + 1e-10) - sigma
    ab = np.cumprod(t, -1)
    ab = np.concatenate([np.ones_like(ab[..., :1]), ab[..., :-1]], -1)
    rgb = np.sum(sigma * ab * feat, -1)
    out = np.transpose(rgb, (0, 2, 1))[:, None]
    s = (out - out.mean()) / (np.std(out, ddof=1) + EPS)
    return ((s - s.min() + EPS) / (s.max() - s.min() + EPS)).astype(np.float32)


def kernel(image3d, R, T):
    if not _axis_aligned(R, T):
        return _numpy_fallback(image3d, R, T)
    out, _ = _run(image3d, R, T, trace=False)
    return out


# revision 17
# speedup vs baseline: 1.8063x; 1.0300x over previous
"""Trainium2 Bass kernel for DirectVolumeRenderer (axis-aligned camera).

Factorization (per depth p, camera R=I so sample coords are separable):
    trilinear(vol) = z-lerp of 2 slices -> two matmuls with the SAME tent
    matrix  A_p[v,q] = relu(1 - |v - (a_p + s_p*q)|):
        T1   = Zp^T @ A_p          (contract y)
        feat = A_p^T @ T1          (contract x) -> image in [px,py] layout
Transmittance Gamma_k is data-independent (density is the constant 0.1,
geometry fixed): on sigma_k's support every earlier sigma_j was fully
inside its own valid square, so Gamma_k == gamma_k =
prod_{j<k}(1 - 0.1*az_j), a host scalar.  Host-side folding leaves the
device a single bf16 matmul chain per depth:

  zm_k = bf16( (0.1*gamma_k*az_k/S_c) * (wz0*vol[z0] + wz1*vol[z1]) )
      -- z-lerp merged on host, compositing scalar folded in, S_c = the
         core's max scalar (host multiplies the partial back by S_c)
  At_k = bf16( av_k(q) * relu(1 - |v - ic_k(q)|) )
      -- av (validity weight sum) folded into columns covers the sigma
         field, so the compositing weight is a pure scalar

Per depth: mm1 (4 bf16 matmuls) -> psT; one ACT/DVE-split bf16 cast;
mm2 (4 bf16 matmuls) accumulated into a single rgb PSUM bank across all
depths (start only on the bank's first matmul -- start=True marks the
WHOLE 2KB bank pending-zero, so later region-first writes land on
pending bytes and write fresh).  No compositing chain, no identity
matmul, no residual streams; fp8 was tried and is NOT faster here (PE
fp8 DoubleRow doubles contraction depth, not column rate, so the
fp8+residual scheme costs 2x this PE time at worse accuracy).
Per-core HBM traffic: nd*256KB + 128KB out."""
import os
import sys
import numpy as np

for _p in ("/opt/trn_rl_repo", "/root/.axon_site/_ro/trn_rl_repo"):
    if os.path.isdir(_p) and _p not in sys.path:
        sys.path.insert(0, _p)

IMG = 256
NPTS = 320
MIN_D, MAX_D = 2.0, 6.0
FOCAL = 2.0
DENSITY = 0.1
EPS = 1e-8
N_CORES = 8
ND = 6           # depths per core (8*ND depths kept in front-to-back order)


# ----------------------------------------------------------------------------
# host-side geometry
# ----------------------------------------------------------------------------

def _geometry(T):
    """Per-depth separable sampling params (f64). Requires R=I and Tx==Ty."""
    Tx, Ty, Tz = float(T[0]), float(T[1]), float(T[2])
    vox = 3.0 / 256.0
    half = vox * 255.0 * 0.5
    depths = np.linspace(MIN_D, MAX_D, NPTS)
    c = depths * 127.5 / (2.0 * half)
    s = c * (2.0 / 255.0)
    a = 127.5 - c - Tx * 127.5 / half
    iz = 127.5 * ((depths - Tz) / half + 1.0)
    z0 = np.floor(iz).astype(np.int64)
    fz = iz - z0
    z1 = z0 + 1
    wz0 = np.where((z0 >= 0) & (z0 < 256), 1.0 - fz, 0.0)
    wz1 = np.where((z1 >= 0) & (z1 < 256), fz, 0.0)
    az = wz0 + wz1
    return dict(s=s, a=a, z0=z0, z1=z1, wz0=wz0, wz1=wz1, az=az, active=az > 0)


def _blk(m):
    """(256, N) f32 -> (128, 2*N) with row p = [t0 block | t1 block]."""
    n = m.shape[1]
    return np.ascontiguousarray(
        m.reshape(2, 128, n).transpose(1, 0, 2).reshape(128, 2 * n))


def _host_inputs(vol, T):
    """Build the 8 per-core input maps + per-core output scales.

    vol: (256,256,256) f32 (z,y,x)."""
    import ml_dtypes
    bf = ml_dtypes.bfloat16
    g = _geometry(T)
    act = np.nonzero(g["active"])[0]

    # gamma_k = prod_{j<k} (1 - 0.1*az_j): global transmittance scalars
    cfac = 1.0 - DENSITY * g["az"]
    gam = np.ones(NPTS)
    gam[1:] = np.cumprod(cfac)[:-1]

    nk = min(N_CORES * ND, len(act))
    act = act[:nk]
    nd = ND
    wk = DENSITY * gam * g["az"]          # per-depth compositing scalar

    qrow = np.arange(IMG, dtype=np.float64)
    vgrid = np.arange(256, dtype=np.float64)

    in_maps, s_cores = [], []
    for cidx in range(N_CORES):
        ks = [int(act[i]) for i in range(cidx * nd, min((cidx + 1) * nd, nk))]
        s_c = max(float(wk[p]) for p in ks) if ks else 1.0
        stream = np.zeros((128, nd * 1024), bf)
        for j, p in enumerate(ks):
            zz0 = min(max(int(g["z0"][p]), 0), 255)
            zz1 = min(max(int(g["z1"][p]), 0), 255)
            zm = (g["wz0"][p] * vol[zz0].astype(np.float64)
                  + g["wz1"][p] * vol[zz1].astype(np.float64))
            zm = (zm * (wk[p] / s_c)).astype(np.float32)
            # tent A[v, q] with av (validity weight sum) folded into columns
            ic = g["a"][p] + g["s"][p] * qrow
            c0 = np.floor(ic)
            fc = ic - c0
            av = (np.where((c0 >= 0) & (c0 < 256), 1.0 - fc, 0.0)
                  + np.where((c0 + 1 >= 0) & (c0 + 1 < 256), fc, 0.0))
            A = np.clip(1.0 - np.abs(vgrid[:, None] - ic[None, :]), 0.0, None)
            base = j * 1024
            stream[:, base:base + 512] = _blk(zm)
            stream[:, base + 512:base + 1024] = _blk(
                (A * av[None, :]).astype(np.float32))
        in_maps.append({"stream": stream})
        s_cores.append(s_c)
    return in_maps, nd, s_cores


# ----------------------------------------------------------------------------
# device program
# ----------------------------------------------------------------------------

_NC_CACHE = {}


def _patch_walrus_flags():
    """Cap the walrus semaphore file: the NEFF epilogue clears every
    allocatable semaphore one-by-one on the engines (~8us for 256), so a
    smaller file directly shrinks the fixed teardown."""
    from concourse import bass_utils
    if getattr(bass_utils, "_sem_cap_patched", False):
        return
    orig = bass_utils.run_command

    def run_command(argv, **kwargs):
        if argv and "walrus_driver" in str(argv[0]):
            argv = list(argv) + [f"--max-sem-num={_MAX_SEMS}"]
        return orig(argv, **kwargs)

    bass_utils.run_command = run_command
    bass_utils._sem_cap_patched = True


_MAX_SEMS = 64
N_WARM = 30      # PE warmup matmuls to ramp the clock during the DMA prologue


def _build_nc(nd):
    import concourse.bass as bass
    import concourse.tile as tile
    from concourse import bacc, mybir
    from contextlib import ExitStack

    _patch_walrus_flags()

    dt = mybir.dt.float32
    dh = mybir.dt.bfloat16
    dhalf = mybir.dt.float16
    AF = mybir.ActivationFunctionType

    nc = bacc.Bacc(None, num_devices=N_CORES)
    str_d = nc.dram_tensor("stream", [128, nd * 1024], dh, kind="ExternalInput")
    out_d = nc.dram_tensor("out", [256, 256], dhalf, kind="ExternalOutput")

    CH = 2                       # depths per DMA chunk
    NCH = (nd + CH - 1) // CH
    PREF = 2

    with tile.TileContext(nc) as tc, ExitStack() as ctx:
        slp = ctx.enter_context(tc.tile_pool(name="slp", bufs=PREF + 1))
        work = ctx.enter_context(tc.tile_pool(name="work", bufs=3))
        epil = ctx.enter_context(tc.tile_pool(name="epil", bufs=1))
        psT = ctx.enter_context(
            tc.tile_pool(name="psT", bufs=3, space=bass.MemorySpace.PSUM))
        psacc = ctx.enter_context(
            tc.tile_pool(name="psacc", bufs=1, space=bass.MemorySpace.PSUM))
        pswarm = ctx.enter_context(
            tc.tile_pool(name="pswarm", bufs=1, space=bass.MemorySpace.PSUM))

        chunks = [None] * NCH

        def issue_chunk(j):
            ndep = min(CH, nd - CH * j)
            t = slp.tile([128, ndep * 4, 256], dh, tag="chunk")
            nc.sync.dma_start(
                t[:].rearrange("p t x -> p (t x)"),
                str_d[:, j * CH * 1024:j * CH * 1024 + ndep * 1024])
            chunks[j] = t

        for j in range(min(PREF, NCH)):
            issue_chunk(j)

        # PE warmup: ramp the tensor-engine clock (1.2 -> 2.4 GHz needs
        # ~3us of continuous work) while the first chunk DMA is in flight
        if N_WARM:
            warm = epil.tile([128, 128], dh, tag="warm")
            nc.vector.memset(warm[:], 0.0)
            wps = pswarm.tile([128, 128], dt, tag="warmps")
            for _ in range(N_WARM):
                nc.tensor.matmul(wps[:], warm[:], warm[:],
                                 start=True, stop=True, skip_group_check=True)

        rgbps = psacc.tile([128, 512], dt, tag="rgb")
        t1S = [None] * nd

        def views(k):
            o = (k % CH) * 4
            t = chunks[k // CH]
            return t[:, o:o + 2, :], t[:, o + 2:o + 4, :]

        def emit_mm1(k):
            # T1[x, py] = sum_y zm[y, x] At[y, py], y = t*128 + part.
            # start=True marks the WHOLE 2KB psum bank pending-zero, so
            # only the bank's first matmul sets it; later first-writes to
            # other byte ranges land on pending bytes (= write fresh).
            zm, at = views(k)
            ps = psT.tile([128, 512], dt, tag="t1s")
            for b in (0, 1):
                for t in (0, 1):
                    nc.tensor.matmul(ps[:, 256 * b:256 * (b + 1)],
                                     zm[:, t, 128 * b:128 * (b + 1)],
                                     at[:, t, :],
                                     start=(b == 0 and t == 0),
                                     stop=(b == 1 and t == 1))
            t1S[k] = ps

        emit_mm1(0)

        for k in range(nd):
            if k % CH == 0 and k // CH + PREF < NCH:
                issue_chunk(k // CH + PREF)

            ps = t1S[k]
            # --- T1 cast PSUM f32 -> SBUF bf16, split ACT | DVE ---
            t1sb = work.tile([128, 2, 256], dh, tag="t1sb")
            nc.scalar.activation(t1sb[:, 0, :], ps[:, 0:256], AF.Copy)
            nc.vector.tensor_copy(t1sb[:, 1, :], ps[:, 256:512])

            # --- keep PE ahead: mm1 for the next depth first ---
            if k + 1 < nd:
                emit_mm1(k + 1)

            # --- mm2: rgb[px, py] += sum_x At[x, px] T1[x, py] ---
            _, at = views(k)
            for m in (0, 1):
                for t in (0, 1):
                    nc.tensor.matmul(rgbps[:, 256 * m:256 * (m + 1)],
                                     at[:, t, 128 * m:128 * (m + 1)],
                                     t1sb[:, t, :],
                                     start=(k == 0 and m == 0 and t == 0),
                                     stop=(k == nd - 1 and m == 1 and t == 1),
                                     skip_group_check=True)

        # ---- per-core partial out; host scales by S_c, sums, normalizes ----
        outsb = epil.tile([128, 512], dhalf, tag="outsb")
        nc.scalar.activation(outsb[:, 0:256], rgbps[:, 0:256], AF.Copy)
        nc.vector.tensor_copy(outsb[:, 256:512], rgbps[:, 256:512])
        nc.sync.dma_start(out_d[:].rearrange("(b p) y -> p b y", p=128),
                          outsb[:].rearrange("p (b y) -> p b y", b=2))
    return nc


# ----------------------------------------------------------------------------
# entry points
# ----------------------------------------------------------------------------

def _axis_aligned(R, T):
    return (np.allclose(np.asarray(R[0]), np.eye(3), atol=1e-6)
            and abs(float(T[0][0]) - float(T[0][1])) < 1e-12)


class _CachedSpmd:
    """Compile the PJRT executable once; repeat calls only transfer + exec."""

    def __init__(self, nc, n_cores):
        import jax
        from concourse import mybir
        from concourse.bass2jax import (_bass_exec_p, install_neuronx_cc_hook,
                                        partition_id_tensor)
        from jax.experimental.shard_map import shard_map
        from jax.sharding import Mesh, PartitionSpec
        install_neuronx_cc_hook()
        self.jax = jax
        self.n_cores = n_cores
        pname = nc.partition_id_tensor.name if nc.partition_id_tensor else None
        in_names, out_names, out_avals, zero_outs = [], [], [], []
        for alloc in nc.m.functions[0].allocations:
            if not isinstance(alloc, mybir.MemoryLocationSet):
                continue
            name = alloc.memorylocations[0].name
            if alloc.kind == "ExternalInput":
                if name != pname:
                    in_names.append(name)
            elif alloc.kind == "ExternalOutput":
                shape = tuple(alloc.tensor_shape)
                dtype = mybir.dt.np(alloc.dtype)
                out_names.append(name)
                out_avals.append(jax.core.ShapedArray(shape, dtype))
                zero_outs.append(np.zeros(shape, dtype))
        self.in_names, self.out_names = in_names, out_names
        self.out_avals, self.zero_outs = out_avals, zero_outs
        n_params, n_outs = len(in_names), len(out_names)
        all_in = list(in_names) + list(out_names)
        if pname is not None:
            all_in.append(pname)

        def _body(*args):
            operands = list(args)
            if pname is not None:
                operands.append(partition_id_tensor())
            outs = _bass_exec_p.bind(
                *operands, out_avals=tuple(out_avals), in_names=tuple(all_in),
                out_names=tuple(out_names), lowering_input_output_aliases=(),
                sim_require_finite=True, sim_require_nnan=True, nc=nc)
            return tuple(outs)

        devices = jax.devices()[:n_cores]
        mesh = Mesh(np.asarray(devices), ("core",))
        in_specs = (PartitionSpec("core"),) * (n_params + n_outs)
        out_specs = (PartitionSpec("core"),) * n_outs
        self.fn = jax.jit(shard_map(_body, mesh=mesh, in_specs=in_specs,
                                    out_specs=out_specs, check_rep=False),
                          keep_unused=True)
        self._dev_zeros = [jax.device_put(np.zeros(
            (n_cores * z.shape[0], *z.shape[1:]), z.dtype)) for z in zero_outs]

    def run(self, in_maps):
        jax = self.jax
        concat = [np.concatenate([np.asarray(in_maps[c][nm])
                                  for c in range(self.n_cores)], axis=0)
                  for nm in self.in_names]
        outs = self.fn(*concat, *self._dev_zeros)
        jax.block_until_ready(outs)
        return [{nm: np.asarray(outs[i]).reshape(
                    self.n_cores, *self.out_avals[i].shape)[c]
                 for i, nm in enumerate(self.out_names)}
                for c in range(self.n_cores)]


_RUNNER_CACHE = {}


def _run(image3d, R, T, trace=False):
    vol = np.ascontiguousarray(np.asarray(image3d, np.float32)[0, 0])
    in_maps, nd, s_cores = _host_inputs(vol, np.asarray(T, np.float64)[0])
    _patch_walrus_flags()
    if nd not in _NC_CACHE:
        nc = _build_nc(nd)
        nc.finalize()
        _NC_CACHE[nd] = nc
    nc = _NC_CACHE[nd]
    if id(nc) not in _RUNNER_CACHE:
        _RUNNER_CACHE[id(nc)] = _CachedSpmd(nc, N_CORES)
    results = _RUNNER_CACHE[id(nc)].run(in_maps)
    # unshard: the S_c-scaled depth-sharded partials sum to the full image
    acc = np.zeros((256, 256), np.float64)
    for c in range(N_CORES):
        acc += s_cores[c] * np.asarray(results[c]["out"], np.float32)
    # normalization (exact reference formula)
    s = (acc - acc.mean()) / (np.std(acc, ddof=1) + EPS)
    out = ((s - s.min() + EPS) / (s.max() - s.min() + EPS)).astype(np.float32)
    return out[None, None], results


def _numpy_fallback(image3d, R, T):
    """Direct port of the reference for non-axis-aligned cameras."""
    image3d = np.asarray(image3d, np.float32)
    R = np.asarray(R, np.float32); T = np.asarray(T, np.float32)
    B, C, D, H, W = image3d.shape
    vol = image3d[:, 0]
    vox = 3.0 / max(C, D)
    yg, xg = np.meshgrid(np.linspace(-1, 1, IMG), np.linspace(-1, 1, IMG),
                         indexing='ij')
    depths = np.linspace(MIN_D, MAX_D, NPTS)
    pcam = np.stack([xg[..., None] * depths / FOCAL,
                     yg[..., None] * depths / FOCAL,
                     np.broadcast_to(depths, (IMG, IMG, NPTS))], -1)
    v = pcam[None] - T[:, None, None, None, :]
    pw = np.einsum('bhwpj,bkj->bhwpk', v, R)
    half = np.array([vox * (W - 1) / 2, vox * (H - 1) / 2, vox * (D - 1) / 2])
    local = pw / half

    def tri(voln, pts):
        ix = (pts[..., 0] + 1) * .5 * (W - 1)
        iy = (pts[..., 1] + 1) * .5 * (H - 1)
        iz = (pts[..., 2] + 1) * .5 * (D - 1)
        out = np.zeros(ix.shape, np.float32)
        x0, y0, z0 = np.floor(ix), np.floor(iy), np.floor(iz)
        fx, fy, fz = ix - x0, iy - y0, iz - z0
        for zi, wz in ((z0, 1 - fz), (z0 + 1, fz)):
            for yi, wy in ((y0, 1 - fy), (y0 + 1, fy)):
                for xi, wx in ((x0, 1 - fx), (x0 + 1, fx)):
                    valid = ((xi >= 0) & (xi < W) & (yi >= 0) & (yi < H)
                             & (zi >= 0) & (zi < D))
                    vv = voln[np.clip(zi, 0, D - 1).astype(int),
                              np.clip(yi, 0, H - 1).astype(int),
                              np.clip(xi, 0, W - 1).astype(int)]
                    out += np.where(valid, vv * (wz * wy * wx), 0).astype(np.float32)
        return out

    feat = np.stack([tri(vol[b], local[b]) for b in range(B)])
    sigma = DENSITY * np.stack([tri(np.ones((D, H, W), np.float32), local[b])
                                for b in range(B)])
    t = (1.0 + 1e-10) - sigma
    ab = np.cumprod(t, -1)
    ab = np.concatenate([np.ones_like(ab[..., :1]), ab[..., :-1]], -1)
    rgb = np.sum(sigma * ab * feat, -1)
    out = np.transpose(rgb, (0, 2, 1))[:, None]
    s = (out - out.mean()) / (np.std(out, ddof=1) + EPS)
    return ((s - s.min() + EPS) / (s.max() - s.min() + EPS)).astype(np.float32)


def kernel(image3d, R, T):
    if not _axis_aligned(R, T):
        return _numpy_fallback(image3d, R, T)
    out, _ = _run(image3d, R, T, trace=False)
    return out
